# revision 13
# baseline (speedup 1.0000x reference)
"""Trainium2 Bass kernel for nn_ChannelizedLinearCompression.

Computation (fp32 reference):
    h1      = relu(einsum('bcn,cnh->bch', x, W1) + b1)   # [B, C, H]
    h2      = relu(einsum('bch,chk->bck', h1, W2) + b2)  # [B, C, 10]
    scalars = einsum('bck,ck->bc', h2, W3) + b3          # [B, C]
    out     = relu(scalars @ Wf1 + bf1) @ Wf2 + bf2      # [B, 16]

Sharding: 2 batch groups x 4 channel groups over 8 cores. Each core gets
x^T[c_loc, N, b_loc] (host-transposed so every big DMA is contiguous) and
computes scalars^T[c_loc, b_loc] on device; the tiny final MLP (0.003% of
the FLOPs) runs on host.

Device per-core dataflow (v2):
  The H=286 output rows split into h-chunks [128, 128, 30]. A naive
  M-major loop wastes 26% of PE columns on the 30-row tail. Instead the
  k-sweep interleaves all 3 local channels and col-tiles the three
  30-row tails into ONE psum bank via tile_position=(0, 32c), so they
  stream concurrently on different PE column groups.

  PSUM budget (8 banks): 6 full-M banks (3c x 2 h-chunks) + 1 trio bank
  + 1 stage-2/3 bank. That forces the batch dim into two j-passes of
  F=512. W1 is loaded once into SBUF (just-in-time groups of 4 k-chunks
  during pass j0, host-shuffled so rows are 2.2KB) and reused in j1.
  x tiles stream per (j, k, c) on both HWDGE queues (sync + scalar).
"""

import os
from contextlib import ExitStack

import numpy as np

import concourse.bass as bass
import concourse.tile as tile
from concourse import bacc, mybir
from concourse.bass_utils import run_bass_kernel_spmd
from concourse._compat import get_trn_type

# Problem shapes (hardcoded; kernel.py must be self-contained).
B, C, N = 2048, 12, 8192
H, MID = 286, 10
FINAL_HIDDEN, LOWDIM = 30, 16
BG, CG = 2, 4  # batch groups x channel groups = 8 cores
B_LOC, C_LOC = B // BG, C // CG

NK = N // 128          # 64 contraction chunks
KG = 4                 # W1 k-chunks per DMA group (2.2KB rows)
NKG = NK // KG         # 16 W1 DMA groups per channel
F = 512                # j-pass width (one PSUM bank of fp32)
NJ = B_LOC // F        # 2 j-passes
HFULL = [(0, 128), (128, 128)]  # full-M h-chunks
H3_0, H3_S = 256, 30            # the 30-row tail chunk

F16 = mybir.dt.float16
F32 = mybir.dt.float32
RELU = mybir.ActivationFunctionType.Relu
IDENT = mybir.ActivationFunctionType.Identity

LAST = {}  # introspection for test.py (exec_time_ns etc.); harness ignores


def build_nc():
    nc = bacc.Bacc(get_trn_type() or "TRN2", target_bir_lowering=False)
    xt = nc.declare_dram_parameter("xt", [C_LOC, N, B_LOC], F16, isOutput=False)
    w1 = nc.declare_dram_parameter("w1", [C_LOC, NKG, 128, KG * H], F16,
                                   isOutput=False)
    b1 = nc.declare_dram_parameter("b1", [C_LOC, H, 1], F32, isOutput=False)
    w2 = nc.declare_dram_parameter("w2", [C_LOC, H, MID], F16, isOutput=False)
    b2 = nc.declare_dram_parameter("b2", [C_LOC, MID, 1], F32, isOutput=False)
    w3 = nc.declare_dram_parameter("w3", [C_LOC, MID, 1], F16, isOutput=False)
    b3 = nc.declare_dram_parameter("b3", [C_LOC, 1, 1], F32, isOutput=False)
    out = nc.declare_dram_parameter("out", [C_LOC, B_LOC], F32, isOutput=True)

    with tile.TileContext(nc) as tc, ExitStack() as ctx:
        xp = ctx.enter_context(tc.tile_pool(name="xp", bufs=36))
        wp = ctx.enter_context(tc.tile_pool(name="wp", bufs=1))
        hp = ctx.enter_context(tc.tile_pool(name="hp", bufs=2))
        sp = ctx.enter_context(tc.tile_pool(name="sp", bufs=1))
        op = ctx.enter_context(tc.tile_pool(name="op", bufs=2))
        pp = ctx.enter_context(
            tc.tile_pool(name="pp", bufs=1, space=bass.MemorySpace.PSUM)
        )

        dmae = [nc.sync, nc.scalar]  # the two HWDGE queues

        # W1 resident tiles, one per channel, filled JIT during pass j0.
        w1r = [wp.tile([128, NK * H], F16, tag=f"w1r{c}", name=f"w1r{c}")
               for c in range(C_LOC)]

        # Kick off the first x tiles and first W1 groups before anything
        # else so the PE can start ASAP.
        xtt = {}
        PRE = 10  # x prefetch depth in (j, k) steps
        W1PRE = 3  # W1 prefetch lead in KG-groups

        def fetch_x(s):
            if s >= NJ * NK:
                return
            j, k = divmod(s, NK)
            for c in range(C_LOC):
                t = xp.tile([128, F], F16, tag="xtt", name=f"xtt{j}_{k}_{c}")
                dmae[(s * C_LOC + c) % 2].dma_start(
                    t[:, :], xt[c, k * 128:(k + 1) * 128, j * F:(j + 1) * F])
                xtt[(j, k, c)] = t

        def fetch_w1(kk):
            # Split W1 between the gpsimd SWDGE queue and the two HWDGE
            # queues so the j0 pass (which carries all of W1 on top of x)
            # isn't bottlenecked on the HW queues.
            if kk >= NKG:
                return
            for c in range(C_LOC):
                if (kk + c) % 2 == 0:
                    eng = nc.gpsimd
                else:
                    eng = dmae[(kk + c) // 2 % 2]
                eng.dma_start(w1r[c][:, kk * KG * H:(kk + 1) * KG * H],
                              w1[c, kk])

        for s in range(2):
            fetch_x(s)
        fetch_w1(0)
        for s in range(2, PRE):
            fetch_x(s)
        for kk in range(1, W1PRE):
            fetch_w1(kk)

        # Small per-channel tensors (biases, W2, W3).
        b1t = [[sp.tile([128, 1], F32, tag=f"b1t{c}_{i}", name=f"b1t{c}_{i}")
                for i in range(2)] for c in range(C_LOC)]
        b1t3 = sp.tile([96, 1], F32, tag="b1t3", name="b1t3")
        w2t = [[sp.tile([128, MID], F16, tag=f"w2t{c}_{i}", name=f"w2t{c}_{i}")
                for i in range(2)] for c in range(C_LOC)]
        w2t3 = sp.tile([96, MID], F16, tag="w2t3", name="w2t3")
        w3t = [sp.tile([MID, 1], F16, tag=f"w3t{c}", name=f"w3t{c}")
               for c in range(C_LOC)]
        b2t = [sp.tile([MID, 1], F32, tag=f"b2t{c}", name=f"b2t{c}")
               for c in range(C_LOC)]
        b3t = [sp.tile([1, 1], F32, tag=f"b3t{c}", name=f"b3t{c}")
               for c in range(C_LOC)]
        for c in range(C_LOC):
            for i, (h0, hs) in enumerate(HFULL):
                nc.gpsimd.dma_start(b1t[c][i][:hs, :], b1[c, h0:h0 + hs, :])
                nc.gpsimd.dma_start(w2t[c][i][:hs, :], w2[c, h0:h0 + hs, :])
            nc.gpsimd.dma_start(b1t3[32 * c:32 * c + H3_S, :],
                                b1[c, H3_0:H3_0 + H3_S, :])
            nc.gpsimd.dma_start(w2t3[32 * c:32 * c + H3_S, :],
                                w2[c, H3_0:H3_0 + H3_S, :])
            nc.gpsimd.dma_start(w3t[c][:, :], w3[c])
            nc.gpsimd.dma_start(b2t[c][:, :], b2[c])
            nc.gpsimd.dma_start(b3t[c][:, :], b3[c])

        sct = [op.tile([1, B_LOC], F32, tag=f"sct{c}", name=f"sct{c}")
               for c in range(C_LOC)]

        for j in range(NJ):
            # Stage 1: psum accumulation over the k sweep, channels
            # interleaved; 30-row tails col-tiled into one trio bank.
            ps = [[pp.tile([128, F], F32, tag=f"ps{c}_{i}",
                           name=f"ps{j}_{c}_{i}")
                   for i in range(2)] for c in range(C_LOC)]
            ps3 = pp.tile([96, F], F32, tag="ps3", name=f"ps3_{j}")
            for k in range(NK):
                # prefetch: x tiles PRE steps ahead, W1 groups in pass j0
                if j == 0 and k % KG == 0:
                    fetch_w1(k // KG + W1PRE)
                fetch_x(j * NK + k + PRE)
                start, stop = k == 0, k == NK - 1
                for c in range(C_LOC):
                    xk = xtt[(j, k, c)]
                    for i, (h0, hs) in enumerate(HFULL):
                        nc.tensor.matmul(
                            ps[c][i][:, :],
                            w1r[c][:, k * H + h0:k * H + h0 + hs],
                            xk[:, :],
                            start=start, stop=stop,
                        )
                for c in range(C_LOC):
                    nc.tensor.matmul(
                        ps3[32 * c:32 * c + H3_S, :],
                        w1r[c][:, k * H + H3_0:k * H + H3_0 + H3_S],
                        xtt[(j, k, c)][:, :],
                        start=start, stop=stop,
                        tile_position=(0, 32 * c),
                    )
                for c in range(C_LOC):
                    del xtt[(j, k, c)]

            # Evict h1 (relu + bias) per channel, then immediately chain
            # that channel's stage 2+3 so the tail stays short.
            h1t = [[hp.tile([128, F], F16, tag=f"h1t{c}_{i}",
                            name=f"h1t{j}_{c}_{i}") for i in range(2)]
                   for c in range(C_LOC)]
            h1t3 = hp.tile([96, F], F16, tag="h1t3", name=f"h1t3_{j}")
            for c in range(C_LOC):
                for i, (h0, hs) in enumerate(HFULL):
                    nc.scalar.activation(h1t[c][i][:hs, :], ps[c][i][:hs, :],
                                         RELU, bias=b1t[c][i][:hs, :])
                nc.scalar.activation(
                    h1t3[32 * c:32 * c + H3_S, :],
                    ps3[32 * c:32 * c + H3_S, :],
                    RELU, bias=b1t3[32 * c:32 * c + H3_S, :])

                p2 = pp.tile([MID, F], F32, tag="p23", name=f"p2_{j}_{c}")
                for i, (h0, hs) in enumerate(HFULL):
                    nc.tensor.matmul(p2[:, :], w2t[c][i][:hs, :],
                                     h1t[c][i][:hs, :],
                                     start=(i == 0), stop=False)
                nc.tensor.matmul(p2[:, :], w2t3[32 * c:32 * c + H3_S, :],
                                 h1t3[32 * c:32 * c + H3_S, :],
                                 start=False, stop=True)
                h2t = op.tile([MID, F], F16, tag="h2t", name=f"h2t{j}_{c}")
                nc.scalar.activation(h2t[:, :], p2[:, :], RELU,
                                     bias=b2t[c][:, :])
                p3 = pp.tile([1, F], F32, tag="p23", name=f"p3_{j}_{c}")
                nc.tensor.matmul(p3[:, :], w3t[c][:, :], h2t[:, :],
                                 start=True, stop=True)
                nc.scalar.activation(sct[c][0:1, j * F:(j + 1) * F],
                                     p3[:, :], IDENT, bias=b3t[c][:, :])
                nc.sync.dma_start(out[c:c + 1, j * F:(j + 1) * F],
                                  sct[c][0:1, j * F:(j + 1) * F])

    nc.compile()
    return nc


_NC_CACHE = {}


def _get_nc():
    if "nc" not in _NC_CACHE:
        _NC_CACHE["nc"] = build_nc()
    return _NC_CACHE["nc"]


def _f16(arr):
    return np.ascontiguousarray(arr, dtype=np.float16)


def _transpose_shard(xs):
    """[b_loc, c_loc, n] -> contiguous [c_loc, n, b_loc]."""
    try:
        import torch
        try:
            torch.set_num_threads(max(os.cpu_count() or 1, 1))
        except Exception:
            pass
        return torch.from_numpy(np.ascontiguousarray(xs)).permute(
            1, 2, 0).contiguous().numpy()
    except ImportError:
        return np.ascontiguousarray(np.transpose(xs, (1, 2, 0)))


def _shuffle_w1(w1c):
    """[c_loc, N, H] -> [c_loc, NKG, 128, KG*H] with
    out[c, kk, p, i*H:(i+1)*H] = w1c[c, (kk*KG+i)*128 + p, :]."""
    r = w1c.reshape(C_LOC, NKG, KG, 128, H)
    return np.ascontiguousarray(r.transpose(0, 1, 3, 2, 4)).reshape(
        C_LOC, NKG, 128, KG * H)


def kernel(x, W1, b1, W2, b2, W3, b3, Wf1, bf1, Wf2, bf2):
    x = np.asarray(x, dtype=np.float32)
    W1 = np.asarray(W1, dtype=np.float32)
    b1 = np.asarray(b1, dtype=np.float32)
    W2 = np.asarray(W2, dtype=np.float32)
    b2 = np.asarray(b2, dtype=np.float32)
    W3 = np.asarray(W3, dtype=np.float32)
    b3 = np.asarray(b3, dtype=np.float32)

    nc = _get_nc()

    # cast before transposing so the shuffle moves half the bytes
    x = _f16(x)

    in_maps = []
    for ib in range(BG):
        bs = slice(ib * B_LOC, (ib + 1) * B_LOC)
        for ic in range(CG):
            cs = slice(ic * C_LOC, (ic + 1) * C_LOC)
            in_maps.append({
                "xt": _f16(_transpose_shard(x[bs, cs, :])),
                "w1": _shuffle_w1(_f16(W1[cs])),
                "b1": np.ascontiguousarray(b1[cs])[:, :, None],
                "w2": _f16(W2[cs]),
                "b2": np.ascontiguousarray(b2[cs])[:, :, None],
                "w3": _f16(W3[cs])[:, :, None],
                "b3": np.ascontiguousarray(b3[cs])[:, None, None],
            })

    res = run_bass_kernel_spmd(nc, in_maps, list(range(BG * CG)))
    LAST["exec_time_ns"] = res.exec_time_ns
    LAST["results"] = res

    scalars = np.empty((B, C), np.float32)
    idx = 0
    for ib in range(BG):
        bs = slice(ib * B_LOC, (ib + 1) * B_LOC)
        for ic in range(CG):
            cs = slice(ic * C_LOC, (ic + 1) * C_LOC)
            scalars[bs, cs] = res.results[idx]["out"].T
            idx += 1

    # Final tiny MLP (C -> 30 -> lowdim) on host in fp32.
    h = np.maximum(scalars @ np.asarray(Wf1, np.float32)
                   + np.asarray(bf1, np.float32), 0.0)
    return (h @ np.asarray(Wf2, np.float32)
            + np.asarray(bf2, np.float32)).astype(np.float32)


# revision 19
# speedup vs baseline: 1.0876x; 1.0876x over previous
"""Trainium2 Bass kernel for nn_ChannelizedLinearCompression.

Computation (fp32 reference):
    h1      = relu(einsum('bcn,cnh->bch', x, W1) + b1)   # [B, C, H]
    h2      = relu(einsum('bch,chk->bck', h1, W2) + b2)  # [B, C, 10]
    scalars = einsum('bck,ck->bc', h2, W3) + b3          # [B, C]
    out     = relu(scalars @ Wf1 + bf1) @ Wf2 + bf2      # [B, 16]

Sharding: 2 batch groups x 4 channel groups over 8 cores. Each core gets
x^T[c_loc, N, b_loc] (host-transposed so every big DMA is contiguous) and
computes scalars^T[c_loc, b_loc] on device; the tiny final MLP (0.003% of
the FLOPs) runs on host.

Device per-core dataflow (v2):
  The H=286 output rows split into h-chunks [128, 128, 30]. A naive
  M-major loop wastes 26% of PE columns on the 30-row tail. Instead the
  k-sweep interleaves all 3 local channels and col-tiles the three
  30-row tails into ONE psum bank via tile_position=(0, 32c), so they
  stream concurrently on different PE column groups.

  PSUM budget (8 banks): 6 full-M banks (3c x 2 h-chunks) + 1 trio bank
  + 1 stage-2/3 bank. That forces the batch dim into two j-passes of
  F=512. W1 is loaded once into SBUF (just-in-time groups of 4 k-chunks
  during pass j0, host-shuffled so rows are 2.2KB) and reused in j1.
  x tiles stream per (j, k, c) on both HWDGE queues (sync + scalar).
"""

import os
from contextlib import ExitStack

import numpy as np

import concourse.bass as bass
import concourse.tile as tile
from concourse import bacc, mybir
from concourse.bass_utils import run_bass_kernel_spmd
from concourse._compat import get_trn_type

# Problem shapes (hardcoded; kernel.py must be self-contained).
B, C, N = 2048, 12, 8192
H, MID = 286, 10
FINAL_HIDDEN, LOWDIM = 30, 16
BG, CG = 2, 4  # batch groups x channel groups = 8 cores
B_LOC, C_LOC = B // BG, C // CG

NK = N // 128          # 64 contraction chunks
KG = 4                 # W1 k-chunks per DMA group (2.2KB rows)
NKG = NK // KG         # 16 W1 DMA groups per channel
F = 512                # j-pass width (one PSUM bank of fp32)
NJ = B_LOC // F        # 2 j-passes
HFULL = [(0, 128), (128, 128)]  # full-M h-chunks
H3_0, H3_S = 256, 30            # the 30-row tail chunk

F16 = mybir.dt.float16
F32 = mybir.dt.float32
RELU = mybir.ActivationFunctionType.Relu
IDENT = mybir.ActivationFunctionType.Identity

LAST = {}  # introspection for test.py (exec_time_ns etc.); harness ignores


def build_nc():
    nc = bacc.Bacc(get_trn_type() or "TRN2", target_bir_lowering=False)
    xt = nc.declare_dram_parameter("xt", [N, C_LOC, B_LOC], F16, isOutput=False)
    w1 = nc.declare_dram_parameter("w1", [C_LOC, NKG, 128, KG * H], F16,
                                   isOutput=False)
    b1 = nc.declare_dram_parameter("b1", [C_LOC, H, 1], F32, isOutput=False)
    w2 = nc.declare_dram_parameter("w2", [C_LOC, H, MID], F16, isOutput=False)
    b2 = nc.declare_dram_parameter("b2", [C_LOC, MID, 1], F32, isOutput=False)
    w3 = nc.declare_dram_parameter("w3", [C_LOC, MID, 1], F16, isOutput=False)
    b3 = nc.declare_dram_parameter("b3", [C_LOC, 1, 1], F32, isOutput=False)
    out = nc.declare_dram_parameter("out", [C_LOC, B_LOC], F32, isOutput=True)

    with tile.TileContext(nc) as tc, ExitStack() as ctx:
        xp = ctx.enter_context(tc.tile_pool(name="xp", bufs=14))
        wp = ctx.enter_context(tc.tile_pool(name="wp", bufs=1))
        hp = ctx.enter_context(tc.tile_pool(name="hp", bufs=2))
        sp = ctx.enter_context(tc.tile_pool(name="sp", bufs=1))
        op = ctx.enter_context(tc.tile_pool(name="op", bufs=2))
        pp = ctx.enter_context(
            tc.tile_pool(name="pp", bufs=1, space=bass.MemorySpace.PSUM)
        )

        dmae = [nc.sync, nc.scalar]  # the two HWDGE queues

        # W1 resident tiles, one per channel, filled JIT during pass j0.
        w1r = [wp.tile([128, NK * H], F16, tag=f"w1r{c}", name=f"w1r{c}")
               for c in range(C_LOC)]

        # Kick off the first x tiles and first W1 groups before anything
        # else so the PE can start ASAP.
        xtt = {}
        PRE = 10  # x prefetch depth in (j, k) steps
        W1PRE = 3  # W1 prefetch lead in KG-groups

        def fetch_x(s):
            # One DMA per (j, k) step covering all 3 channels: fewer,
            # bigger descriptors and fewer engine-queue instructions.
            if s >= NJ * NK:
                return
            j, k = divmod(s, NK)
            t = xp.tile([128, C_LOC, F], F16, tag="xtt", name=f"xtt{j}_{k}")
            dmae[s % 2].dma_start(
                t[:, :, :], xt[k * 128:(k + 1) * 128, :, j * F:(j + 1) * F])
            for c in range(C_LOC):
                xtt[(j, k, c)] = t[:, c, :]

        def fetch_w1(kk):
            # Split W1 between the gpsimd SWDGE queue and the two HWDGE
            # queues so the j0 pass (which carries all of W1 on top of x)
            # isn't bottlenecked on the HW queues.
            if kk >= NKG:
                return
            for c in range(C_LOC):
                i = kk * C_LOC + c
                if i % 2 == 0:
                    eng = nc.gpsimd
                else:
                    eng = dmae[(i // 2) % 2]
                eng.dma_start(w1r[c][:, kk * KG * H:(kk + 1) * KG * H],
                              w1[c, kk])

        for s in range(2):
            fetch_x(s)
        fetch_w1(0)
        for s in range(2, PRE):
            fetch_x(s)
        for kk in range(1, W1PRE):
            fetch_w1(kk)

        # Small per-channel tensors (biases, W2, W3).
        b1t = [[sp.tile([128, 1], F32, tag=f"b1t{c}_{i}", name=f"b1t{c}_{i}")
                for i in range(2)] for c in range(C_LOC)]
        b1t3 = sp.tile([96, 1], F32, tag="b1t3", name="b1t3")
        w2t = [[sp.tile([128, MID], F16, tag=f"w2t{c}_{i}", name=f"w2t{c}_{i}")
                for i in range(2)] for c in range(C_LOC)]
        w2t3 = sp.tile([96, MID], F16, tag="w2t3", name="w2t3")
        w3t = [sp.tile([MID, 1], F16, tag=f"w3t{c}", name=f"w3t{c}")
               for c in range(C_LOC)]
        b2t = [sp.tile([MID, 1], F32, tag=f"b2t{c}", name=f"b2t{c}")
               for c in range(C_LOC)]
        b3t = [sp.tile([1, 1], F32, tag=f"b3t{c}", name=f"b3t{c}")
               for c in range(C_LOC)]
        for c in range(C_LOC):
            for i, (h0, hs) in enumerate(HFULL):
                nc.gpsimd.dma_start(b1t[c][i][:hs, :], b1[c, h0:h0 + hs, :])
                nc.gpsimd.dma_start(w2t[c][i][:hs, :], w2[c, h0:h0 + hs, :])
            nc.gpsimd.dma_start(b1t3[32 * c:32 * c + H3_S, :],
                                b1[c, H3_0:H3_0 + H3_S, :])
            nc.gpsimd.dma_start(w2t3[32 * c:32 * c + H3_S, :],
                                w2[c, H3_0:H3_0 + H3_S, :])
            nc.gpsimd.dma_start(w3t[c][:, :], w3[c])
            nc.gpsimd.dma_start(b2t[c][:, :], b2[c])
            nc.gpsimd.dma_start(b3t[c][:, :], b3[c])

        sct = [op.tile([1, B_LOC], F32, tag=f"sct{c}", name=f"sct{c}")
               for c in range(C_LOC)]

        for j in range(NJ):
            # Stage 1: psum accumulation over the k sweep, channels
            # interleaved; 30-row tails col-tiled into one trio bank.
            ps = [[pp.tile([128, F], F32, tag=f"ps{c}_{i}",
                           name=f"ps{j}_{c}_{i}")
                   for i in range(2)] for c in range(C_LOC)]
            ps3 = pp.tile([96, F], F32, tag="ps3", name=f"ps3_{j}")
            for k in range(NK):
                # prefetch: x tiles PRE steps ahead, W1 groups in pass j0
                if j == 0 and k % KG == 0:
                    fetch_w1(k // KG + W1PRE)
                fetch_x(j * NK + k + PRE)
                start, stop = k == 0, k == NK - 1
                for c in range(C_LOC):
                    xk = xtt[(j, k, c)]
                    for i, (h0, hs) in enumerate(HFULL):
                        nc.tensor.matmul(
                            ps[c][i][:, :],
                            w1r[c][:, k * H + h0:k * H + h0 + hs],
                            xk[:, :],
                            start=start, stop=stop,
                        )
                for c in range(C_LOC):
                    nc.tensor.matmul(
                        ps3[32 * c:32 * c + H3_S, :],
                        w1r[c][:, k * H + H3_0:k * H + H3_0 + H3_S],
                        xtt[(j, k, c)][:, :],
                        start=start, stop=stop,
                        tile_position=(0, 32 * c),
                    )
                for c in range(C_LOC):
                    del xtt[(j, k, c)]

            # Evict h1 (relu + bias) per channel, then immediately chain
            # that channel's stage 2+3 so the tail stays short.
            h1t = [[hp.tile([128, F], F16, tag=f"h1t{c}_{i}",
                            name=f"h1t{j}_{c}_{i}") for i in range(2)]
                   for c in range(C_LOC)]
            h1t3 = hp.tile([96, F], F16, tag="h1t3", name=f"h1t3_{j}")
            # Evictions run on the DVE (not the scalar engine) so they never
            # queue behind the scalar engine's DMA issues at pass boundaries.
            for c in range(C_LOC):
                for i, (h0, hs) in enumerate(HFULL):
                    nc.vector.tensor_scalar(
                        h1t[c][i][:hs, :], ps[c][i][:hs, :],
                        b1t[c][i][:hs, :], 0.0,
                        mybir.AluOpType.add, mybir.AluOpType.max)
                nc.vector.tensor_scalar(
                    h1t3[32 * c:32 * c + H3_S, :],
                    ps3[32 * c:32 * c + H3_S, :],
                    b1t3[32 * c:32 * c + H3_S, :], 0.0,
                    mybir.AluOpType.add, mybir.AluOpType.max)

                p2 = pp.tile([MID, F], F32, tag="p23", name=f"p2_{j}_{c}")
                for i, (h0, hs) in enumerate(HFULL):
                    nc.tensor.matmul(p2[:, :], w2t[c][i][:hs, :],
                                     h1t[c][i][:hs, :],
                                     start=(i == 0), stop=False)
                nc.tensor.matmul(p2[:, :], w2t3[32 * c:32 * c + H3_S, :],
                                 h1t3[32 * c:32 * c + H3_S, :],
                                 start=False, stop=True)
                h2t = op.tile([MID, F], F16, tag="h2t", name=f"h2t{j}_{c}")
                nc.vector.tensor_scalar(h2t[:, :], p2[:, :], b2t[c][:, :],
                                        0.0, mybir.AluOpType.add,
                                        mybir.AluOpType.max)
                p3 = pp.tile([1, F], F32, tag="p23", name=f"p3_{j}_{c}")
                nc.tensor.matmul(p3[:, :], w3t[c][:, :], h2t[:, :],
                                 start=True, stop=True)
                nc.vector.tensor_scalar(sct[c][0:1, j * F:(j + 1) * F],
                                        p3[:, :], b3t[c][:, :], None,
                                        mybir.AluOpType.add)
                nc.sync.dma_start(out[c:c + 1, j * F:(j + 1) * F],
                                  sct[c][0:1, j * F:(j + 1) * F])

    nc.compile()
    return nc


_NC_CACHE = {}


def _get_nc():
    if "nc" not in _NC_CACHE:
        _NC_CACHE["nc"] = build_nc()
    return _NC_CACHE["nc"]


def _f16(arr):
    return np.ascontiguousarray(arr, dtype=np.float16)


def _transpose_shard(xs):
    """[b_loc, c_loc, n] -> contiguous [n, c_loc, b_loc]."""
    try:
        import torch
        try:
            torch.set_num_threads(max(os.cpu_count() or 1, 1))
        except Exception:
            pass
        return torch.from_numpy(np.ascontiguousarray(xs)).permute(
            2, 1, 0).contiguous().numpy()
    except ImportError:
        return np.ascontiguousarray(np.transpose(xs, (2, 1, 0)))


def _shuffle_w1(w1c):
    """[c_loc, N, H] -> [c_loc, NKG, 128, KG*H] with
    out[c, kk, p, i*H:(i+1)*H] = w1c[c, (kk*KG+i)*128 + p, :]."""
    r = w1c.reshape(C_LOC, NKG, KG, 128, H)
    return np.ascontiguousarray(r.transpose(0, 1, 3, 2, 4)).reshape(
        C_LOC, NKG, 128, KG * H)


def kernel(x, W1, b1, W2, b2, W3, b3, Wf1, bf1, Wf2, bf2):
    x = np.asarray(x, dtype=np.float32)
    W1 = np.asarray(W1, dtype=np.float32)
    b1 = np.asarray(b1, dtype=np.float32)
    W2 = np.asarray(W2, dtype=np.float32)
    b2 = np.asarray(b2, dtype=np.float32)
    W3 = np.asarray(W3, dtype=np.float32)
    b3 = np.asarray(b3, dtype=np.float32)

    nc = _get_nc()

    # cast before transposing so the shuffle moves half the bytes
    x = _f16(x)

    in_maps = []
    for ib in range(BG):
        bs = slice(ib * B_LOC, (ib + 1) * B_LOC)
        for ic in range(CG):
            cs = slice(ic * C_LOC, (ic + 1) * C_LOC)
            in_maps.append({
                "xt": _f16(_transpose_shard(x[bs, cs, :])),
                "w1": _shuffle_w1(_f16(W1[cs])),
                "b1": np.ascontiguousarray(b1[cs])[:, :, None],
                "w2": _f16(W2[cs]),
                "b2": np.ascontiguousarray(b2[cs])[:, :, None],
                "w3": _f16(W3[cs])[:, :, None],
                "b3": np.ascontiguousarray(b3[cs])[:, None, None],
            })

    res = run_bass_kernel_spmd(nc, in_maps, list(range(BG * CG)))
    LAST["exec_time_ns"] = res.exec_time_ns
    LAST["results"] = res

    scalars = np.empty((B, C), np.float32)
    idx = 0
    for ib in range(BG):
        bs = slice(ib * B_LOC, (ib + 1) * B_LOC)
        for ic in range(CG):
            cs = slice(ic * C_LOC, (ic + 1) * C_LOC)
            scalars[bs, cs] = res.results[idx]["out"].T
            idx += 1

    # Final tiny MLP (C -> 30 -> lowdim) on host in fp32.
    h = np.maximum(scalars @ np.asarray(Wf1, np.float32)
                   + np.asarray(bf1, np.float32), 0.0)
    return (h @ np.asarray(Wf2, np.float32)
            + np.asarray(bf2, np.float32)).astype(np.float32)


# revision 22
# speedup vs baseline: 1.0883x; 1.0006x over previous
"""Trainium2 Bass kernel for nn_ChannelizedLinearCompression.

Computation (fp32 reference):
    h1      = relu(einsum('bcn,cnh->bch', x, W1) + b1)   # [B, C, H]
    h2      = relu(einsum('bch,chk->bck', h1, W2) + b2)  # [B, C, 10]
    scalars = einsum('bck,ck->bc', h2, W3) + b3          # [B, C]
    out     = relu(scalars @ Wf1 + bf1) @ Wf2 + bf2      # [B, 16]

Sharding: 2 batch groups x 4 channel groups over 8 cores. Each core gets
x^T[c_loc, N, b_loc] (host-transposed so every big DMA is contiguous) and
computes scalars^T[c_loc, b_loc] on device; the tiny final MLP (0.003% of
the FLOPs) runs on host.

Device per-core dataflow (v2):
  The H=286 output rows split into h-chunks [128, 128, 30]. A naive
  M-major loop wastes 26% of PE columns on the 30-row tail. Instead the
  k-sweep interleaves all 3 local channels and col-tiles the three
  30-row tails into ONE psum bank via tile_position=(0, 32c), so they
  stream concurrently on different PE column groups.

  PSUM budget (8 banks): 6 full-M banks (3c x 2 h-chunks) + 1 trio bank
  + 1 stage-2/3 bank. That forces the batch dim into two j-passes of
  F=512. W1 is loaded once into SBUF (just-in-time groups of 4 k-chunks
  during pass j0, host-shuffled so rows are 2.2KB) and reused in j1.
  x tiles stream per (j, k, c) on both HWDGE queues (sync + scalar).
"""

import os
from contextlib import ExitStack

import numpy as np

import concourse.bass as bass
import concourse.tile as tile
from concourse import bacc, mybir
from concourse.bass_utils import run_bass_kernel_spmd
from concourse._compat import get_trn_type

# Problem shapes (hardcoded; kernel.py must be self-contained).
B, C, N = 2048, 12, 8192
H, MID = 286, 10
FINAL_HIDDEN, LOWDIM = 30, 16
BG, CG = 2, 4  # batch groups x channel groups = 8 cores
B_LOC, C_LOC = B // BG, C // CG

NK = N // 128          # 64 contraction chunks
KG = 4                 # W1 k-chunks per DMA group (2.2KB rows)
NKG = NK // KG         # 16 W1 DMA groups per channel
F = 512                # j-pass width (one PSUM bank of fp32)
NJ = B_LOC // F        # 2 j-passes
HFULL = [(0, 128), (128, 128)]  # full-M h-chunks
H3_0, H3_S = 256, 30            # the 30-row tail chunk

F16 = mybir.dt.float16
F32 = mybir.dt.float32
RELU = mybir.ActivationFunctionType.Relu
IDENT = mybir.ActivationFunctionType.Identity

LAST = {}  # introspection for test.py (exec_time_ns etc.); harness ignores


def build_nc():
    nc = bacc.Bacc(get_trn_type() or "TRN2", target_bir_lowering=False)
    xt = nc.declare_dram_parameter("xt", [N, C_LOC, B_LOC], F16, isOutput=False)
    w1 = nc.declare_dram_parameter("w1", [C_LOC, NKG, 128, KG * H], F16,
                                   isOutput=False)
    b1 = nc.declare_dram_parameter("b1", [C_LOC, H, 1], F32, isOutput=False)
    w2 = nc.declare_dram_parameter("w2", [C_LOC, H, MID], F16, isOutput=False)
    b2 = nc.declare_dram_parameter("b2", [C_LOC, MID, 1], F32, isOutput=False)
    w3 = nc.declare_dram_parameter("w3", [C_LOC, MID, 1], F16, isOutput=False)
    b3 = nc.declare_dram_parameter("b3", [C_LOC, 1, 1], F32, isOutput=False)
    out = nc.declare_dram_parameter("out", [C_LOC, B_LOC], F32, isOutput=True)

    with tile.TileContext(nc) as tc, ExitStack() as ctx:
        xp = ctx.enter_context(tc.tile_pool(name="xp", bufs=16))
        wp = ctx.enter_context(tc.tile_pool(name="wp", bufs=1))
        hp = ctx.enter_context(tc.tile_pool(name="hp", bufs=2))
        sp = ctx.enter_context(tc.tile_pool(name="sp", bufs=1))
        op = ctx.enter_context(tc.tile_pool(name="op", bufs=2))
        pp = ctx.enter_context(
            tc.tile_pool(name="pp", bufs=1, space=bass.MemorySpace.PSUM)
        )

        dmae = [nc.sync, nc.scalar]  # the two HWDGE queues

        # W1 resident tiles, one per channel, filled JIT during pass j0.
        w1r = [wp.tile([128, NK * H], F16, tag=f"w1r{c}", name=f"w1r{c}")
               for c in range(C_LOC)]

        # Kick off the first x tiles and first W1 groups before anything
        # else so the PE can start ASAP.
        xtt = {}
        PRE = 12  # x prefetch depth in (j, k) steps
        W1PRE = 3  # W1 prefetch lead in KG-groups

        def fetch_x(s, split=False):
            # One DMA per (j, k) step covering all 3 channels: fewer,
            # bigger descriptors and fewer engine-queue instructions.
            # split=True issues per-channel DMAs (for fast startup).
            if s >= NJ * NK:
                return
            j, k = divmod(s, NK)
            t = xp.tile([128, C_LOC, F], F16, tag="xtt", name=f"xtt{j}_{k}")
            if split:
                for c in range(C_LOC):
                    dmae[c % 2].dma_start(
                        t[:, c, :],
                        xt[k * 128:(k + 1) * 128, c, j * F:(j + 1) * F])
            else:
                dmae[s % 2].dma_start(
                    t[:, :, :],
                    xt[k * 128:(k + 1) * 128, :, j * F:(j + 1) * F])
            for c in range(C_LOC):
                xtt[(j, k, c)] = t[:, c, :]

        def fetch_w1(kk):
            # W1 rides mostly on the gpsimd SWDGE queue (2/3) so the j0
            # pass (which carries all of W1 on top of x) isn't
            # bottlenecked on the two HWDGE queues.
            if kk >= NKG:
                return
            for c in range(C_LOC):
                i = kk * C_LOC + c
                if i % 3 != 2:
                    eng = nc.gpsimd
                else:
                    eng = dmae[(i // 3) % 2]
                eng.dma_start(w1r[c][:, kk * KG * H:(kk + 1) * KG * H],
                              w1[c, kk])

        fetch_x(0, split=True)
        fetch_w1(0)
        fetch_x(1, split=True)
        for s in range(2, PRE):
            fetch_x(s)
        for kk in range(1, W1PRE):
            fetch_w1(kk)

        # Small per-channel tensors (biases, W2, W3).
        b1t = [[sp.tile([128, 1], F32, tag=f"b1t{c}_{i}", name=f"b1t{c}_{i}")
                for i in range(2)] for c in range(C_LOC)]
        b1t3 = sp.tile([96, 1], F32, tag="b1t3", name="b1t3")
        w2t = [[sp.tile([128, MID], F16, tag=f"w2t{c}_{i}", name=f"w2t{c}_{i}")
                for i in range(2)] for c in range(C_LOC)]
        w2t3 = sp.tile([96, MID], F16, tag="w2t3", name="w2t3")
        w3t = [sp.tile([MID, 1], F16, tag=f"w3t{c}", name=f"w3t{c}")
               for c in range(C_LOC)]
        b2t = [sp.tile([MID, 1], F32, tag=f"b2t{c}", name=f"b2t{c}")
               for c in range(C_LOC)]
        b3t = [sp.tile([1, 1], F32, tag=f"b3t{c}", name=f"b3t{c}")
               for c in range(C_LOC)]
        for c in range(C_LOC):
            for i, (h0, hs) in enumerate(HFULL):
                nc.gpsimd.dma_start(b1t[c][i][:hs, :], b1[c, h0:h0 + hs, :])
                nc.gpsimd.dma_start(w2t[c][i][:hs, :], w2[c, h0:h0 + hs, :])
            nc.gpsimd.dma_start(b1t3[32 * c:32 * c + H3_S, :],
                                b1[c, H3_0:H3_0 + H3_S, :])
            nc.gpsimd.dma_start(w2t3[32 * c:32 * c + H3_S, :],
                                w2[c, H3_0:H3_0 + H3_S, :])
            nc.gpsimd.dma_start(w3t[c][:, :], w3[c])
            nc.gpsimd.dma_start(b2t[c][:, :], b2[c])
            nc.gpsimd.dma_start(b3t[c][:, :], b3[c])

        sct = [op.tile([1, B_LOC], F32, tag=f"sct{c}", name=f"sct{c}")
               for c in range(C_LOC)]

        for j in range(NJ):
            # Stage 1: psum accumulation over the k sweep, channels
            # interleaved; 30-row tails col-tiled into one trio bank.
            ps = [[pp.tile([128, F], F32, tag=f"ps{c}_{i}",
                           name=f"ps{j}_{c}_{i}")
                   for i in range(2)] for c in range(C_LOC)]
            ps3 = pp.tile([96, F], F32, tag="ps3", name=f"ps3_{j}")
            for k in range(NK):
                # prefetch: x tiles PRE steps ahead, W1 groups in pass j0
                if j == 0 and k % KG == 0:
                    fetch_w1(k // KG + W1PRE)
                fetch_x(j * NK + k + PRE)
                start, stop = k == 0, k == NK - 1
                for c in range(C_LOC):
                    xk = xtt[(j, k, c)]
                    for i, (h0, hs) in enumerate(HFULL):
                        nc.tensor.matmul(
                            ps[c][i][:, :],
                            w1r[c][:, k * H + h0:k * H + h0 + hs],
                            xk[:, :],
                            start=start, stop=stop,
                        )
                for c in range(C_LOC):
                    nc.tensor.matmul(
                        ps3[32 * c:32 * c + H3_S, :],
                        w1r[c][:, k * H + H3_0:k * H + H3_0 + H3_S],
                        xtt[(j, k, c)][:, :],
                        start=start, stop=stop,
                        tile_position=(0, 32 * c),
                    )
                for c in range(C_LOC):
                    del xtt[(j, k, c)]

            # Evict h1 (relu + bias) per channel, then immediately chain
            # that channel's stage 2+3 so the tail stays short.
            h1t = [[hp.tile([128, F], F16, tag=f"h1t{c}_{i}",
                            name=f"h1t{j}_{c}_{i}") for i in range(2)]
                   for c in range(C_LOC)]
            h1t3 = hp.tile([96, F], F16, tag="h1t3", name=f"h1t3_{j}")
            # Evictions run on the DVE (not the scalar engine) so they never
            # queue behind the scalar engine's DMA issues at pass
            # boundaries. In the final pass the scalar engine is idle, so
            # alternate DVE/scalar to halve the tail's eviction latency.
            last = j == NJ - 1
            ecnt = [0]

            def evict(dst, src, bias):
                if last and ecnt[0] % 2 == 1:
                    nc.scalar.activation(dst, src, RELU, bias=bias)
                else:
                    nc.vector.tensor_scalar(dst, src, bias, 0.0,
                                            mybir.AluOpType.add,
                                            mybir.AluOpType.max)
                ecnt[0] += 1

            for c in range(C_LOC):
                for i, (h0, hs) in enumerate(HFULL):
                    evict(h1t[c][i][:hs, :], ps[c][i][:hs, :],
                          b1t[c][i][:hs, :])
                evict(h1t3[32 * c:32 * c + H3_S, :],
                      ps3[32 * c:32 * c + H3_S, :],
                      b1t3[32 * c:32 * c + H3_S, :])

                # In the last pass, borrow the just-freed stage-1 banks for
                # stage 2/3 so the per-channel chains overlap.
                p2 = pp.tile([MID, F], F32,
                             tag=f"ps{c}_0" if last else "p23",
                             name=f"p2_{j}_{c}")
                for i, (h0, hs) in enumerate(HFULL):
                    nc.tensor.matmul(p2[:, :], w2t[c][i][:hs, :],
                                     h1t[c][i][:hs, :],
                                     start=(i == 0), stop=False)
                nc.tensor.matmul(p2[:, :], w2t3[32 * c:32 * c + H3_S, :],
                                 h1t3[32 * c:32 * c + H3_S, :],
                                 start=False, stop=True)
                h2t = op.tile([MID, F], F16, tag="h2t", name=f"h2t{j}_{c}")
                evict(h2t[:, :], p2[:, :], b2t[c][:, :])
                p3 = pp.tile([1, F], F32,
                             tag=f"ps{c}_1" if last else "p23",
                             name=f"p3_{j}_{c}")
                nc.tensor.matmul(p3[:, :], w3t[c][:, :], h2t[:, :],
                                 start=True, stop=True)
                if last:
                    nc.scalar.activation(sct[c][0:1, j * F:(j + 1) * F],
                                         p3[:, :], IDENT, bias=b3t[c][:, :])
                else:
                    nc.vector.tensor_scalar(sct[c][0:1, j * F:(j + 1) * F],
                                            p3[:, :], b3t[c][:, :], None,
                                            mybir.AluOpType.add)
                nc.sync.dma_start(out[c:c + 1, j * F:(j + 1) * F],
                                  sct[c][0:1, j * F:(j + 1) * F])

    nc.compile()
    return nc


_NC_CACHE = {}


def _get_nc():
    if "nc" not in _NC_CACHE:
        _NC_CACHE["nc"] = build_nc()
    return _NC_CACHE["nc"]


def _f16(arr):
    return np.ascontiguousarray(arr, dtype=np.float16)


def _transpose_shard(xs):
    """[b_loc, c_loc, n] -> contiguous [n, c_loc, b_loc]."""
    try:
        import torch
        try:
            torch.set_num_threads(max(os.cpu_count() or 1, 1))
        except Exception:
            pass
        return torch.from_numpy(np.ascontiguousarray(xs)).permute(
            2, 1, 0).contiguous().numpy()
    except ImportError:
        return np.ascontiguousarray(np.transpose(xs, (2, 1, 0)))


def _shuffle_w1(w1c):
    """[c_loc, N, H] -> [c_loc, NKG, 128, KG*H] with
    out[c, kk, p, i*H:(i+1)*H] = w1c[c, (kk*KG+i)*128 + p, :]."""
    r = w1c.reshape(C_LOC, NKG, KG, 128, H)
    return np.ascontiguousarray(r.transpose(0, 1, 3, 2, 4)).reshape(
        C_LOC, NKG, 128, KG * H)


def kernel(x, W1, b1, W2, b2, W3, b3, Wf1, bf1, Wf2, bf2):
    x = np.asarray(x, dtype=np.float32)
    W1 = np.asarray(W1, dtype=np.float32)
    b1 = np.asarray(b1, dtype=np.float32)
    W2 = np.asarray(W2, dtype=np.float32)
    b2 = np.asarray(b2, dtype=np.float32)
    W3 = np.asarray(W3, dtype=np.float32)
    b3 = np.asarray(b3, dtype=np.float32)

    nc = _get_nc()

    # cast before transposing so the shuffle moves half the bytes
    x = _f16(x)

    in_maps = []
    for ib in range(BG):
        bs = slice(ib * B_LOC, (ib + 1) * B_LOC)
        for ic in range(CG):
            cs = slice(ic * C_LOC, (ic + 1) * C_LOC)
            in_maps.append({
                "xt": _f16(_transpose_shard(x[bs, cs, :])),
                "w1": _shuffle_w1(_f16(W1[cs])),
                "b1": np.ascontiguousarray(b1[cs])[:, :, None],
                "w2": _f16(W2[cs]),
                "b2": np.ascontiguousarray(b2[cs])[:, :, None],
                "w3": _f16(W3[cs])[:, :, None],
                "b3": np.ascontiguousarray(b3[cs])[:, None, None],
            })

    res = run_bass_kernel_spmd(nc, in_maps, list(range(BG * CG)))
    LAST["exec_time_ns"] = res.exec_time_ns
    LAST["results"] = res

    scalars = np.empty((B, C), np.float32)
    idx = 0
    for ib in range(BG):
        bs = slice(ib * B_LOC, (ib + 1) * B_LOC)
        for ic in range(CG):
            cs = slice(ic * C_LOC, (ic + 1) * C_LOC)
            scalars[bs, cs] = res.results[idx]["out"].T
            idx += 1

    # Final tiny MLP (C -> 30 -> lowdim) on host in fp32.
    h = np.maximum(scalars @ np.asarray(Wf1, np.float32)
                   + np.asarray(bf1, np.float32), 0.0)
    return (h @ np.asarray(Wf2, np.float32)
            + np.asarray(bf2, np.float32)).astype(np.float32)


# revision 24
# speedup vs baseline: 1.1803x; 1.0845x over previous
"""Trainium2 Bass kernel for nn_ChannelizedLinearCompression.

Computation (fp32 reference):
    h1      = relu(einsum('bcn,cnh->bch', x, W1) + b1)   # [B, C, H]
    h2      = relu(einsum('bch,chk->bck', h1, W2) + b2)  # [B, C, 10]
    scalars = einsum('bck,ck->bc', h2, W3) + b3          # [B, C]
    out     = relu(scalars @ Wf1 + bf1) @ Wf2 + bf2      # [B, 16]

Sharding: 2 batch groups x 4 channel groups over 8 cores. Each core gets
x^T[c_loc, N, b_loc] (host-transposed so every big DMA is contiguous) and
computes scalars^T[c_loc, b_loc] on device; the tiny final MLP (0.003% of
the FLOPs) runs on host.

Device dataflow (v6, x-stationary / b-major):
  Stage-1 matmuls use x tiles as the STATIONARY operand ([128n, 128b]
  slices, always full 128x128) and W1 chunks as the MOVING operand
  ([128n, 288h]).  The psum output [128b, 288h*fp32] fits one bank, so
  the eight b-chunks of b_loc=1024 occupy exactly the 8 psum banks and a
  channel is ONE k-sweep: every W1 chunk is DMA'd and used exactly once.
  That makes HBM demand uniform (~335 GB/s) instead of front-loaded,
  which a 2-pass h-major design cannot achieve (W1 would all be needed
  in the first pass: >400 GB/s vs the ~355 GB/s per-core ceiling).

  The per-h bias rides in as a final K=1 matmul against a ones-row.
  Each bank then drains through a per-bank pipeline in the bank's SPARE
  psum columns (the accumulator uses 1152B of the 2048B bank):
    relu-evict (DVE, b-major fp16) -> PE transpose (3 h-chunks, via
    identity) -> copy to SBUF h-major (ACT) -> stage-2 mms (F=128)
    -> h2 relu (DVE) -> stage-3 mm -> +b3 (DVE) -> out DMA.
  These posts run on the PE between channel sweeps, so the PE never
  idles long enough to drop the HAM clock gate (K=8/8 -> 4/8 at ~3.4us).
"""

import os
from contextlib import ExitStack

import numpy as np

import concourse.bass as bass
import concourse.tile as tile
from concourse import bacc, mybir
from concourse.bass_utils import run_bass_kernel_spmd
from concourse._compat import get_trn_type

# Problem shapes (hardcoded; kernel.py must be self-contained).
B, C, N = 2048, 12, 8192
H, MID = 286, 10
FINAL_HIDDEN, LOWDIM = 30, 16
BG, CG = 2, 4  # batch groups x channel groups = 8 cores
B_LOC, C_LOC = B // BG, C // CG

H2 = 288               # H padded to a multiple of 32 (zero cols)
NK = N // 128          # 64 contraction chunks
KG = 8                 # W1 k-chunks per DMA group (4.6KB rows)
NG = NK // KG          # 8 W1 DMA groups per channel
NJB = B_LOC // 128     # 8 b-chunks = 8 psum banks
HCH = [(0, 128), (128, 128), (256, 32)]  # h-chunks for transpose/stage2
TP0 = 320              # start col (fp32 elems) of the spare psum window

F16 = mybir.dt.float16
F32 = mybir.dt.float32
COPY = mybir.ActivationFunctionType.Copy
ADD = mybir.AluOpType.add
MAX = mybir.AluOpType.max

LAST = {}  # introspection for test.py (exec_time_ns etc.); harness ignores


def build_nc():
    nc = bacc.Bacc(get_trn_type() or "TRN2", target_bir_lowering=False)
    xt = nc.declare_dram_parameter("xt", [C_LOC, N, B_LOC], F16, isOutput=False)
    w1 = nc.declare_dram_parameter("w1", [C_LOC, NG, 128, KG * H2], F16,
                                   isOutput=False)
    b1m = nc.declare_dram_parameter("b1m", [C_LOC, 1, H2], F16, isOutput=False)
    eye = nc.declare_dram_parameter("eye", [128, 128], F16, isOutput=False)
    w2 = nc.declare_dram_parameter("w2", [C_LOC, H2, MID], F16, isOutput=False)
    b2 = nc.declare_dram_parameter("b2", [C_LOC, MID, 1], F32, isOutput=False)
    w3 = nc.declare_dram_parameter("w3", [C_LOC, MID, 1], F16, isOutput=False)
    b3 = nc.declare_dram_parameter("b3", [C_LOC, 1, 1], F32, isOutput=False)
    out = nc.declare_dram_parameter("out", [C_LOC, B_LOC], F32, isOutput=True)

    with tile.TileContext(nc) as tc, ExitStack() as ctx:
        xp = ctx.enter_context(tc.tile_pool(name="xp", bufs=16))
        wp = ctx.enter_context(tc.tile_pool(name="wp", bufs=4))
        hb = ctx.enter_context(tc.tile_pool(name="hb", bufs=10))
        ht = ctx.enter_context(tc.tile_pool(name="ht", bufs=12))
        sp = ctx.enter_context(tc.tile_pool(name="sp", bufs=1))
        op = ctx.enter_context(tc.tile_pool(name="op", bufs=12))
        pp = ctx.enter_context(
            tc.tile_pool(name="pp", bufs=8, space=bass.MemorySpace.PSUM)
        )

        dmae = [nc.sync, nc.scalar]
        wdmae = [nc.sync, nc.scalar, nc.gpsimd]

        xts = {}
        w1g = {}
        PRE = 12   # x prefetch depth in k-steps
        W1PRE = 2  # W1 prefetch lead in KG-groups

        def fetch_x(s):
            if s >= C_LOC * NK:
                return
            c, k = divmod(s, NK)
            t = xp.tile([128, B_LOC], F16, tag="xtt", name=f"xtt{c}_{k}")
            dmae[s % 2].dma_start(t[:, :], xt[c, k * 128:(k + 1) * 128, :])
            xts[(c, k)] = t

        def fetch_w1(gg):
            if gg >= C_LOC * NG:
                return
            c, g = divmod(gg, NG)
            t = wp.tile([128, KG * H2], F16, tag="w1g", name=f"w1g{c}_{g}")
            wdmae[gg % 3].dma_start(t[:, :], w1[c, g])
            w1g[(c, g)] = t

        fetch_x(0)
        fetch_w1(0)
        fetch_x(1)
        fetch_w1(1)
        for s in range(2, PRE):
            fetch_x(s)

        # Small constants (biases, W2, W3, identity, ones-row).
        eye_t = sp.tile([128, 128], F16, tag="eye", name="eye_t")
        nc.gpsimd.dma_start(eye_t[:, :], eye[:, :])
        ones_t = sp.tile([1, 128], F16, tag="ones", name="ones_t")
        nc.vector.memset(ones_t[:, :], 1.0)
        b1mt = [sp.tile([1, H2], F16, tag=f"b1m{c}", name=f"b1mt{c}")
                for c in range(C_LOC)]
        w2t = [[sp.tile([hs, MID], F16, tag=f"w2t{c}_{i}", name=f"w2t{c}_{i}")
                for i, (h0, hs) in enumerate(HCH)] for c in range(C_LOC)]
        w3t = [sp.tile([MID, 1], F16, tag=f"w3t{c}", name=f"w3t{c}")
               for c in range(C_LOC)]
        b2t = [sp.tile([MID, 1], F32, tag=f"b2t{c}", name=f"b2t{c}")
               for c in range(C_LOC)]
        b3t = [sp.tile([1, 1], F32, tag=f"b3t{c}", name=f"b3t{c}")
               for c in range(C_LOC)]
        for c in range(C_LOC):
            nc.gpsimd.dma_start(b1mt[c][:, :], b1m[c])
            for i, (h0, hs) in enumerate(HCH):
                nc.gpsimd.dma_start(w2t[c][i][:, :], w2[c, h0:h0 + hs, :])
            nc.gpsimd.dma_start(w3t[c][:, :], w3[c])
            nc.gpsimd.dma_start(b2t[c][:, :], b2[c])
            nc.gpsimd.dma_start(b3t[c][:, :], b3[c])

        sct = [op.tile([1, B_LOC], F32, tag=f"sct{c}", bufs=1,
                       name=f"sct{c}") for c in range(C_LOC)]

        for c in range(C_LOC):
            # ---- stage-1 k-sweep: 8 psum banks <- all of b_loc ----
            ps = [pp.tile([128, 512], F32, tag="pb", name=f"ps{c}_{jb}")
                  for jb in range(NJB)]
            for k in range(NK):
                fetch_x(c * NK + k + PRE)
                if k % KG == 0:
                    fetch_w1(c * NG + k // KG + W1PRE)
                xk = xts[(c, k)]
                wg = w1g[(c, k // KG)]
                i = k % KG
                rhs = wg[:, i * H2:(i + 1) * H2]
                for jb in range(NJB):
                    nc.tensor.matmul(
                        ps[jb][:, 0:H2],
                        xk[:, jb * 128:(jb + 1) * 128],
                        rhs,
                        start=(k == 0), stop=False,
                    )
                del xts[(c, k)]
                if k % KG == KG - 1:
                    del w1g[(c, k // KG)]

            # bias: += ones.T @ b1 (broadcast along partitions), closes
            # the accumulation group
            for jb in range(NJB):
                nc.tensor.matmul(ps[jb][:, 0:H2], ones_t[:, :],
                                 b1mt[c][:, :], start=False, stop=True)

            # ---- per-bank post pipeline in the spare psum window ----
            h1b = [hb.tile([128, H2], F16, tag="h1b", name=f"h1b{c}_{jb}")
                   for jb in range(NJB)]
            for jb in range(NJB):
                nc.vector.tensor_scalar(h1b[jb][:, :], ps[jb][:, 0:H2],
                                        0.0, None, MAX)

            h1t = {}
            for jb in range(NJB):
                for i, (h0, hs) in enumerate(HCH):
                    tp_out = ps[jb][0:hs, TP0:TP0 + 64].bitcast(F16)
                    nc.tensor.transpose(
                        tp_out,
                        h1b[jb][:, h0:h0 + hs],
                        eye_t[:, :],
                    )
                    t = ht.tile([hs, 128], F16, tag=f"h1t{i}",
                                name=f"h1t{c}_{jb}_{i}")
                    nc.scalar.activation(t[:, :], tp_out, COPY)
                    h1t[(jb, i)] = t

            for jb in range(NJB):
                for i, (h0, hs) in enumerate(HCH):
                    nc.tensor.matmul(
                        ps[jb][0:MID, TP0:TP0 + 128],
                        w2t[c][i][:, :],
                        h1t[(jb, i)][:, :],
                        start=(i == 0), stop=(i == len(HCH) - 1),
                    )
            h2t = {}
            for jb in range(NJB):
                t = op.tile([MID, 128], F16, tag="h2t", name=f"h2t{c}_{jb}")
                nc.vector.tensor_scalar(t[:, :], ps[jb][0:MID, TP0:TP0 + 128],
                                        b2t[c][:, :], 0.0, ADD, MAX)
                h2t[jb] = t
            for jb in range(NJB):
                nc.tensor.matmul(ps[jb][0:1, TP0:TP0 + 128], w3t[c][:, :],
                                 h2t[jb][:, :], start=True, stop=True)
            for jb in range(NJB):
                nc.vector.tensor_scalar(
                    sct[c][0:1, jb * 128:(jb + 1) * 128],
                    ps[jb][0:1, TP0:TP0 + 128],
                    b3t[c][:, :], None, ADD)
            nc.sync.dma_start(out[c:c + 1, :], sct[c][0:1, :])

    nc.compile()
    return nc


_NC_CACHE = {}


def _get_nc():
    if "nc" not in _NC_CACHE:
        _NC_CACHE["nc"] = build_nc()
    return _NC_CACHE["nc"]


def _f16(arr):
    return np.ascontiguousarray(arr, dtype=np.float16)


def _transpose_shard(xs):
    """[b_loc, c_loc, n] -> contiguous [c_loc, n, b_loc]."""
    try:
        import torch
        try:
            torch.set_num_threads(max(os.cpu_count() or 1, 1))
        except Exception:
            pass
        return torch.from_numpy(np.ascontiguousarray(xs)).permute(
            1, 2, 0).contiguous().numpy()
    except ImportError:
        return np.ascontiguousarray(np.transpose(xs, (1, 2, 0)))


def _shuffle_w1(w1c):
    """fp16 [c_loc, N, H] -> [c_loc, NG, 128, KG*H2] with
    out[c, g, p, i*H2+h] = w1pad[c, (g*KG+i)*128 + p, h]."""
    w1p = np.zeros((C_LOC, N, H2), np.float16)
    w1p[:, :, :H] = w1c
    r = w1p.reshape(C_LOC, NG, KG, 128, H2)
    return np.ascontiguousarray(r.transpose(0, 1, 3, 2, 4)).reshape(
        C_LOC, NG, 128, KG * H2)


def kernel(x, W1, b1, W2, b2, W3, b3, Wf1, bf1, Wf2, bf2):
    x = np.asarray(x, dtype=np.float32)
    W1 = np.asarray(W1, dtype=np.float32)
    b1 = np.asarray(b1, dtype=np.float32)
    W2 = np.asarray(W2, dtype=np.float32)
    b2 = np.asarray(b2, dtype=np.float32)
    W3 = np.asarray(W3, dtype=np.float32)
    b3 = np.asarray(b3, dtype=np.float32)

    nc = _get_nc()

    # cast before transposing so the shuffle moves half the bytes
    x = _f16(x)
    eye = np.eye(128, dtype=np.float16)

    in_maps = []
    for ib in range(BG):
        bs = slice(ib * B_LOC, (ib + 1) * B_LOC)
        for ic in range(CG):
            cs = slice(ic * C_LOC, (ic + 1) * C_LOC)
            b1p = np.zeros((C_LOC, 1, H2), np.float16)
            b1p[:, 0, :H] = b1[cs]
            w2p = np.zeros((C_LOC, H2, MID), np.float16)
            w2p[:, :H, :] = W2[cs]
            in_maps.append({
                "xt": _f16(_transpose_shard(x[bs, cs, :])),
                "w1": _shuffle_w1(_f16(W1[cs])),
                "b1m": b1p,
                "eye": eye,
                "w2": w2p,
                "b2": np.ascontiguousarray(b2[cs])[:, :, None],
                "w3": _f16(W3[cs])[:, :, None],
                "b3": np.ascontiguousarray(b3[cs])[:, None, None],
            })

    res = run_bass_kernel_spmd(nc, in_maps, list(range(BG * CG)))
    LAST["exec_time_ns"] = res.exec_time_ns
    LAST["results"] = res

    scalars = np.empty((B, C), np.float32)
    idx = 0
    for ib in range(BG):
        bs = slice(ib * B_LOC, (ib + 1) * B_LOC)
        for ic in range(CG):
            cs = slice(ic * C_LOC, (ic + 1) * C_LOC)
            scalars[bs, cs] = res.results[idx]["out"].T
            idx += 1

    # Final tiny MLP (C -> 30 -> lowdim) on host in fp32.
    h = np.maximum(scalars @ np.asarray(Wf1, np.float32)
                   + np.asarray(bf1, np.float32), 0.0)
    return (h @ np.asarray(Wf2, np.float32)
            + np.asarray(bf2, np.float32)).astype(np.float32)
